# revision 6
# baseline (speedup 1.0000x reference)
"""Trainium2 Bass kernel for the CustomODELoss problem.

Full inputs:
    predicted_solution_batch [4096, 8192] f32
    target_solution_batch    [4096, 8192] f32
    c_input_batch            [4096]       f32
    x_eval_points            [8192]       f32   (uniform grid on [0, 1])

loss = mean((pred - target)^2)
     + mean((pred[r, idx_r] - 1)^2)
     + mean(((pred[r, idx_p] - pred[r, idx_m]) / ((idx_p - idx_m) * dx))^2)
where idx_r = argmin_j |x_j - c_r| (first index on ties).

Sharding: data-parallel over the batch dim, 512 rows per core on 8 cores.
Each core streams its pred/target slice once (memory-bound integral term)
and resolves the per-row grid index + 3-point gather on device via
indirect DMA.  The device emits per-partition partial sums; the host sums
the 8x128 partials and forms the three means.
"""

import numpy as np

import concourse.bacc as bacc
import concourse.bass as bass
import concourse.mybir as mybir
from concourse import tile
from concourse.bass_utils import run_bass_kernel_spmd

F32 = mybir.dt.float32
I32 = mybir.dt.int32

B = 4096
N = 8192
NCORES = 8
BL = B // NCORES          # rows per core = 512
P = 128                   # SBUF partitions
RB = BL // P              # row blocks per core = 4
FT = 4096                 # free-dim tile for the streaming phase
CT = N // FT              # col tiles per row block = 2
NT = RB * CT              # streaming tiles per tensor = 8


def build_nc():
    # Bacc (not plain Bass): its compile pipeline runs
    # generate_event_semaphores, which splits multi-sem waits into separate
    # event instructions — TRN2 allows at most 1 embedded wait per
    # instruction, and walrus codegen rejects the unsplit form.
    nc = bacc.Bacc()

    pred = nc.dram_tensor("pred", [BL, N], F32, kind="ExternalInput")
    targ = nc.dram_tensor("targ", [BL, N], F32, kind="ExternalInput")
    cvec = nc.dram_tensor("cvec", [BL, 1], F32, kind="ExternalInput")
    xev = nc.dram_tensor("xev", [N, 1], F32, kind="ExternalInput")
    dxb = nc.dram_tensor("dxb", [P, 1], F32, kind="ExternalInput")
    partials = nc.dram_tensor("partials", [P, 3], F32, kind="ExternalOutput")

    with tile.TileContext(nc) as tc:
        with (
            tc.tile_pool(name="ppool", bufs=3) as ppool,
            tc.tile_pool(name="tpool", bufs=3) as tpool,
            tc.tile_pool(name="dpool", bufs=2) as dpool,
            tc.tile_pool(name="small", bufs=2) as small,
            tc.tile_pool(name="acc", bufs=1) as acc,
        ):
            # ---- accumulators / constants ----
            parts1 = acc.tile([P, NT], F32)     # per-tile integral partials
            t2cols = acc.tile([P, RB], F32)     # (f(c)-1)^2 per chunk
            t3cols = acc.tile([P, RB], F32)     # f'(c)^2 per chunk
            iota3 = acc.tile([P, 3], F32)
            nc.gpsimd.iota(
                iota3[:], pattern=[[1, 3]], base=0, channel_multiplier=0,
                allow_small_or_imprecise_dtypes=True,
            )
            dx_t = acc.tile([P, 1], F32)
            nc.sync.dma_start(dx_t[:], dxb[:, :])

            # ---- phase A: stream pred/target, accumulate sum((p-t)^2) ----
            for rb in range(RB):
                for ct in range(CT):
                    k = rb * CT + ct
                    rs, cs = rb * P, ct * FT
                    pt = ppool.tile([P, FT], F32)
                    tt = tpool.tile([P, FT], F32)
                    nc.sync.dma_start(pt[:], pred[rs:rs + P, cs:cs + FT])
                    nc.sync.dma_start(tt[:], targ[rs:rs + P, cs:cs + FT])
                    dt = dpool.tile([P, FT], F32)
                    nc.vector.tensor_tensor(
                        out=dt[:], in0=pt[:], in1=tt[:],
                        op=mybir.AluOpType.subtract,
                    )
                    # dt <- dt^2 in place; accum_out = row-sum
                    nc.scalar.activation(
                        out=dt[:], in_=dt[:],
                        func=mybir.ActivationFunctionType.Square,
                        accum_out=parts1[:, k:k + 1],
                    )

            # ---- phase B: per-row index resolve + 3-point gather ----
            for qc in range(RB):
                c_t = small.tile([P, 1], F32)
                nc.sync.dma_start(c_t[:], cvec[qc * P:(qc + 1) * P, :])

                # j0 = int(c * (N-1)); any convert rounding mode keeps
                # |j0 - argmin| <= 1, which the 3-candidate check fixes.
                u = small.tile([P, 1], F32)
                nc.vector.tensor_scalar(
                    out=u[:], in0=c_t[:], scalar1=float(N - 1), scalar2=None,
                    op0=mybir.AluOpType.mult,
                )
                j0i = small.tile([P, 1], I32)
                nc.vector.tensor_copy(out=j0i[:], in_=u[:])
                j0f = small.tile([P, 1], F32)
                nc.vector.tensor_copy(out=j0f[:], in_=j0i[:])
                jcc = small.tile([P, 1], F32)
                nc.vector.tensor_scalar(
                    out=jcc[:], in0=j0f[:], scalar1=1.0, scalar2=float(N - 2),
                    op0=mybir.AluOpType.max, op1=mybir.AluOpType.min,
                )

                # gather x[jc-1 : jc+2]
                s1f = small.tile([P, 1], F32)
                nc.vector.tensor_scalar(
                    out=s1f[:], in0=jcc[:], scalar1=1.0, scalar2=None,
                    op0=mybir.AluOpType.subtract,
                )
                s1i = small.tile([P, 1], I32)
                nc.vector.tensor_copy(out=s1i[:], in_=s1f[:])
                xw = small.tile([P, 3], F32)
                nc.gpsimd.indirect_dma_start(
                    out=xw[:], out_offset=None,
                    in_=xev[:, :],
                    in_offset=bass.IndirectOffsetOnAxis(ap=s1i[:, :1], axis=0),
                )

                # distances |x_k - c| for the 3 candidates
                dsub = small.tile([P, 3], F32)
                nc.vector.tensor_scalar(
                    out=dsub[:], in0=xw[:], scalar1=c_t[:, :1], scalar2=None,
                    op0=mybir.AluOpType.subtract,
                )
                dst = small.tile([P, 3], F32)
                nc.scalar.activation(
                    out=dst[:], in_=dsub[:],
                    func=mybir.ActivationFunctionType.Abs,
                )
                dm, d0, dp = dst[:, 0:1], dst[:, 1:2], dst[:, 2:3]

                # first-argmin among {jc-1, jc, jc+1}
                t1b = small.tile([P, 1], F32)
                nc.vector.tensor_tensor(out=t1b[:], in0=dm, in1=d0,
                                        op=mybir.AluOpType.is_le)
                t2b = small.tile([P, 1], F32)
                nc.vector.tensor_tensor(out=t2b[:], in0=dm, in1=dp,
                                        op=mybir.AluOpType.is_le)
                a_t = small.tile([P, 1], F32)
                nc.vector.tensor_tensor(out=a_t[:], in0=t1b[:], in1=t2b[:],
                                        op=mybir.AluOpType.mult)
                t3b = small.tile([P, 1], F32)
                nc.vector.tensor_tensor(out=t3b[:], in0=d0, in1=dp,
                                        op=mybir.AluOpType.is_le)
                oma = small.tile([P, 1], F32)
                nc.vector.tensor_scalar(
                    out=oma[:], in0=a_t[:], scalar1=-1.0, scalar2=1.0,
                    op0=mybir.AluOpType.mult, op1=mybir.AluOpType.add,
                )
                b_t = small.tile([P, 1], F32)
                nc.vector.tensor_tensor(out=b_t[:], in0=t3b[:], in1=oma[:],
                                        op=mybir.AluOpType.mult)
                # jstar = jc + 1 - 2a - b
                e1 = small.tile([P, 1], F32)
                nc.vector.tensor_scalar(
                    out=e1[:], in0=a_t[:], scalar1=-2.0, scalar2=1.0,
                    op0=mybir.AluOpType.mult, op1=mybir.AluOpType.add,
                )
                e2 = small.tile([P, 1], F32)
                nc.vector.tensor_tensor(out=e2[:], in0=e1[:], in1=b_t[:],
                                        op=mybir.AluOpType.subtract)
                jstar = small.tile([P, 1], F32)
                nc.vector.tensor_tensor(out=jstar[:], in0=jcc[:], in1=e2[:],
                                        op=mybir.AluOpType.add)

                # neighbors and pred-window start
                jm = small.tile([P, 1], F32)
                nc.vector.tensor_scalar(
                    out=jm[:], in0=jstar[:], scalar1=-1.0, scalar2=0.0,
                    op0=mybir.AluOpType.add, op1=mybir.AluOpType.max,
                )
                jp = small.tile([P, 1], F32)
                nc.vector.tensor_scalar(
                    out=jp[:], in0=jstar[:], scalar1=1.0, scalar2=float(N - 1),
                    op0=mybir.AluOpType.add, op1=mybir.AluOpType.min,
                )
                s2 = small.tile([P, 1], F32)
                nc.vector.tensor_scalar(
                    out=s2[:], in0=jm[:], scalar1=float(N - 3), scalar2=None,
                    op0=mybir.AluOpType.min,
                )
                s2i = small.tile([P, 1], I32)
                nc.vector.tensor_copy(out=s2i[:], in_=s2[:])
                rowbase = small.tile([P, 1], I32)
                nc.gpsimd.iota(
                    rowbase[:], pattern=[[0, 1]], base=qc * P * N,
                    channel_multiplier=N,
                )
                offs = small.tile([P, 1], I32)
                nc.vector.tensor_tensor(out=offs[:], in0=rowbase[:], in1=s2i[:],
                                        op=mybir.AluOpType.add)
                pw = small.tile([P, 3], F32)
                nc.gpsimd.indirect_dma_start(
                    out=pw[:], out_offset=None,
                    in_=pred[:, :],
                    in_offset=bass.IndirectOffsetOnAxis(ap=offs[:, :1], axis=1),
                )

                # in-window positions
                p0 = small.tile([P, 1], F32)
                nc.vector.tensor_tensor(out=p0[:], in0=jstar[:], in1=s2[:],
                                        op=mybir.AluOpType.subtract)
                pmp = small.tile([P, 1], F32)
                nc.vector.tensor_tensor(out=pmp[:], in0=jm[:], in1=s2[:],
                                        op=mybir.AluOpType.subtract)
                ppp = small.tile([P, 1], F32)
                nc.vector.tensor_tensor(out=ppp[:], in0=jp[:], in1=s2[:],
                                        op=mybir.AluOpType.subtract)

                # one-hot selects: (iota3 == pos) * window, row-summed
                sc1 = small.tile([P, 3], F32)
                fpc = small.tile([P, 1], F32)
                nc.vector.scalar_tensor_tensor(
                    out=sc1[:], in0=iota3[:], scalar=p0[:, :1], in1=pw[:],
                    op0=mybir.AluOpType.is_equal, op1=mybir.AluOpType.mult,
                    accum_out=fpc[:],
                )
                sc2 = small.tile([P, 3], F32)
                gp = small.tile([P, 1], F32)
                nc.vector.scalar_tensor_tensor(
                    out=sc2[:], in0=iota3[:], scalar=ppp[:, :1], in1=pw[:],
                    op0=mybir.AluOpType.is_equal, op1=mybir.AluOpType.mult,
                    accum_out=gp[:],
                )
                sc3 = small.tile([P, 3], F32)
                gm = small.tile([P, 1], F32)
                nc.vector.scalar_tensor_tensor(
                    out=sc3[:], in0=iota3[:], scalar=pmp[:, :1], in1=pw[:],
                    op0=mybir.AluOpType.is_equal, op1=mybir.AluOpType.mult,
                    accum_out=gm[:],
                )

                # f'(c) = (g_p - g_m) / ((jp - jm) * dx)
                qd = small.tile([P, 1], F32)
                nc.vector.tensor_tensor(out=qd[:], in0=jp[:], in1=jm[:],
                                        op=mybir.AluOpType.subtract)
                den = small.tile([P, 1], F32)
                nc.vector.tensor_scalar(
                    out=den[:], in0=qd[:], scalar1=dx_t[:, :1], scalar2=None,
                    op0=mybir.AluOpType.mult,
                )
                rden = small.tile([P, 1], F32)
                nc.vector.reciprocal(out=rden[:], in_=den[:])
                df = small.tile([P, 1], F32)
                nc.vector.tensor_tensor(out=df[:], in0=gp[:], in1=gm[:],
                                        op=mybir.AluOpType.subtract)
                fpp = small.tile([P, 1], F32)
                nc.vector.tensor_tensor(out=fpp[:], in0=df[:], in1=rden[:],
                                        op=mybir.AluOpType.mult)

                # (f(c) - 1)^2 and f'(c)^2 into per-chunk columns
                fpm1 = small.tile([P, 1], F32)
                nc.vector.tensor_scalar(
                    out=fpm1[:], in0=fpc[:], scalar1=-1.0, scalar2=None,
                    op0=mybir.AluOpType.add,
                )
                nc.scalar.activation(
                    out=t2cols[:, qc:qc + 1], in_=fpm1[:],
                    func=mybir.ActivationFunctionType.Square,
                )
                nc.scalar.activation(
                    out=t3cols[:, qc:qc + 1], in_=fpp[:],
                    func=mybir.ActivationFunctionType.Square,
                )

            # ---- final per-partition reductions + output ----
            p1 = acc.tile([P, 1], F32)
            nc.vector.reduce_sum(out=p1[:], in_=parts1[:],
                                 axis=mybir.AxisListType.X)
            p2 = acc.tile([P, 1], F32)
            nc.vector.reduce_sum(out=p2[:], in_=t2cols[:],
                                 axis=mybir.AxisListType.X)
            p3 = acc.tile([P, 1], F32)
            nc.vector.reduce_sum(out=p3[:], in_=t3cols[:],
                                 axis=mybir.AxisListType.X)
            nc.sync.dma_start(partials[:, 0:1], p1[:])
            nc.sync.dma_start(partials[:, 1:2], p2[:])
            nc.sync.dma_start(partials[:, 2:3], p3[:])

    return nc


_NC_CACHE = None


def _get_nc():
    global _NC_CACHE
    if _NC_CACHE is None:
        nc = build_nc()
        # Bacc runs its compile pipeline (register alloc, sync-wait
        # splitting) in finalize; the PJRT exec path requires it.
        nc.finalize()
        _NC_CACHE = nc
    return _NC_CACHE


def make_in_maps(predicted_solution_batch, target_solution_batch,
                 c_input_batch, x_eval_points):
    pred = np.ascontiguousarray(predicted_solution_batch, dtype=np.float32)
    targ = np.ascontiguousarray(target_solution_batch, dtype=np.float32)
    c = np.ascontiguousarray(c_input_batch, dtype=np.float32)
    x = np.ascontiguousarray(x_eval_points, dtype=np.float32)
    dx = np.float32(x[1]) - np.float32(x[0])
    dxb = np.full((P, 1), dx, dtype=np.float32)
    xev = x.reshape(N, 1)
    in_maps = []
    for i in range(NCORES):
        sl = slice(i * BL, (i + 1) * BL)
        in_maps.append({
            "pred": pred[sl],
            "targ": targ[sl],
            "cvec": c[sl].reshape(BL, 1),
            "xev": xev,
            "dxb": dxb,
        })
    return in_maps


def reduce_partials(results):
    s = np.zeros(3, dtype=np.float64)
    for r in results:
        s += r["partials"].astype(np.float64).sum(axis=0)
    loss = s[0] / (B * N) + s[1] / B + s[2] / B
    return np.float32(loss)


def kernel(predicted_solution_batch, target_solution_batch,
           c_input_batch, x_eval_points):
    nc = _get_nc()
    in_maps = make_in_maps(predicted_solution_batch, target_solution_batch,
                           c_input_batch, x_eval_points)
    res = run_bass_kernel_spmd(nc, in_maps, core_ids=list(range(NCORES)))
    return reduce_partials(res.results)


# revision 12
# speedup vs baseline: 1.2502x; 1.2502x over previous
"""Trainium2 Bass kernel for the CustomODELoss problem.

Full inputs:
    predicted_solution_batch [4096, 8192] f32
    target_solution_batch    [4096, 8192] f32
    c_input_batch            [4096]       f32
    x_eval_points            [8192]       f32   (uniform grid on [0, 1])

loss = mean((pred - target)^2)
     + mean((pred[r, idx_r] - 1)^2)
     + mean(((pred[r, idx_p] - pred[r, idx_m]) / ((idx_p - idx_m) * dx))^2)
where idx_r = argmin_j |x_j - c_r| (first index on ties).

Sharding: data-parallel over the batch dim, 512 rows per core on 8 cores.
Each core streams its pred/target slice once (memory-bound integral term)
and resolves the per-row grid index + 3-point gather on device via
indirect DMA.  The index resolve is exact: a rounding-based candidate
j0 (always within 1 of the true argmin) is corrected by comparing the
f32 distances |x_j - c| of the 3 candidate grid points with the same
first-index tie-break as jnp.argmin.  The device emits per-partition
partial sums; the host sums the 8x128 partials and forms the means.
"""

import numpy as np

import concourse.bacc as bacc
import concourse.bass as bass
import concourse.mybir as mybir
from concourse import tile
from concourse.bass_utils import run_bass_kernel_spmd

F32 = mybir.dt.float32
I32 = mybir.dt.int32
OP = mybir.AluOpType

B = 4096
N = 8192
NCORES = 8
BL = B // NCORES          # rows per core = 512
P = 128                   # SBUF partitions
RB = BL // P              # row groups per partition = 4
FT = 2048                 # free-dim tile for the streaming phase
NT = (BL // P) * (N // FT)  # streaming tile pairs per core = 16


def build_nc(debug=False):
    # Bacc (not plain Bass): its compile pipeline runs
    # generate_event_semaphores, which splits multi-sem waits into separate
    # event instructions — TRN2 allows at most 1 embedded wait per
    # instruction, and walrus codegen rejects the unsplit form.
    nc = bacc.Bacc()

    pred = nc.dram_tensor("pred", [BL, N], F32, kind="ExternalInput")
    targ = nc.dram_tensor("targ", [BL, N], F32, kind="ExternalInput")
    # c per core, reshaped host-side to [128, 4]: row r = p*RB + q
    cvec = nc.dram_tensor("cvec", [P, RB], F32, kind="ExternalInput")
    xev = nc.dram_tensor("xev", [N, 1], F32, kind="ExternalInput")
    dxb = nc.dram_tensor("dxb", [P, 1], F32, kind="ExternalInput")
    partials = nc.dram_tensor("partials", [P, 3], F32, kind="ExternalOutput")
    if debug:
        dbg = nc.dram_tensor("dbg", [P, 44], F32, kind="ExternalOutput")

    def view3(t):  # [128, 12] tile -> [128, 4, 3] AP
        return t[:].rearrange("p (q k) -> p q k", k=3)

    with tile.TileContext(nc) as tc:
        with (
            tc.tile_pool(name="ppool", bufs=6) as ppool,
            tc.tile_pool(name="tpool", bufs=6) as tpool,
            tc.tile_pool(name="dpool", bufs=3) as dpool,
            tc.tile_pool(name="pb", bufs=1) as pb,
        ):
            # ================= phase B: gather terms (one wide pass) ====
            # Emitted first so its long dependency chain (DVE ops +
            # indirect gathers) overlaps the streaming phase below.
            c_t = pb.tile([P, RB], F32)
            nc.sync.dma_start(c_t[:], cvec[:, :])
            dx_t = pb.tile([P, 1], F32)
            nc.sync.dma_start(dx_t[:], dxb[:, :])

            # j0 = int(c * (N-1)); any convert rounding mode keeps
            # |j0 - argmin| <= 1, which the 3-candidate check fixes.
            u = pb.tile([P, RB], F32)
            nc.vector.tensor_scalar(out=u[:], in0=c_t[:], scalar1=float(N - 1),
                                    scalar2=None, op0=OP.mult)
            j0i = pb.tile([P, RB], I32)
            nc.vector.tensor_copy(out=j0i[:], in_=u[:])
            j0f = pb.tile([P, RB], F32)
            nc.vector.tensor_copy(out=j0f[:], in_=j0i[:])
            jcc = pb.tile([P, RB], F32)
            nc.vector.tensor_scalar(out=jcc[:], in0=j0f[:], scalar1=1.0,
                                    scalar2=float(N - 2), op0=OP.max, op1=OP.min)

            # gather x[jc-1 : jc+2] -> [128, 4*3]
            s1f = pb.tile([P, RB], F32)
            nc.vector.tensor_scalar(out=s1f[:], in0=jcc[:], scalar1=1.0,
                                    scalar2=None, op0=OP.subtract)
            s1i = pb.tile([P, RB], I32)
            nc.vector.tensor_copy(out=s1i[:], in_=s1f[:])
            # NOTE: hardware SWDGE honors only ONE offset per partition in an
            # indirect DMA (CoreSim accepts [128, RB] offsets, HW does not) —
            # issue one gather per row-group with [128, 1] offsets.
            xw = pb.tile([P, RB * 3], F32)
            for q in range(RB):
                nc.gpsimd.indirect_dma_start(
                    out=xw[:, 3 * q:3 * q + 3], out_offset=None, in_=xev[:, :],
                    in_offset=bass.IndirectOffsetOnAxis(
                        ap=s1i[:, q:q + 1], axis=0),
                )

            # distances |x_k - c| for the 3 candidates of each row
            dsub = pb.tile([P, RB * 3], F32)
            nc.vector.tensor_tensor(out=view3(dsub), in0=view3(xw),
                                    in1=c_t[:].to_broadcast([P, RB, 3]),
                                    op=OP.subtract)
            dst = pb.tile([P, RB * 3], F32)
            nc.scalar.activation(out=dst[:], in_=dsub[:],
                                 func=mybir.ActivationFunctionType.Abs)
            dm, d0, dp = dst[:, 0::3], dst[:, 1::3], dst[:, 2::3]

            # first-argmin among {jc-1, jc, jc+1}:
            #   a = (dm<=d0)&(dm<=dp); b = (1-a)&(d0<=dp)
            #   jstar = jc + 1 - 2a - b
            t1b = pb.tile([P, RB], F32)
            nc.vector.tensor_tensor(out=t1b[:], in0=dm, in1=d0, op=OP.is_le)
            t2b = pb.tile([P, RB], F32)
            nc.vector.tensor_tensor(out=t2b[:], in0=dm, in1=dp, op=OP.is_le)
            a_t = pb.tile([P, RB], F32)
            nc.vector.tensor_tensor(out=a_t[:], in0=t1b[:], in1=t2b[:],
                                    op=OP.mult)
            t3b = pb.tile([P, RB], F32)
            nc.vector.tensor_tensor(out=t3b[:], in0=d0, in1=dp, op=OP.is_le)
            oma = pb.tile([P, RB], F32)
            nc.vector.tensor_scalar(out=oma[:], in0=a_t[:], scalar1=-1.0,
                                    scalar2=1.0, op0=OP.mult, op1=OP.add)
            b_t = pb.tile([P, RB], F32)
            nc.vector.tensor_tensor(out=b_t[:], in0=t3b[:], in1=oma[:],
                                    op=OP.mult)
            e1 = pb.tile([P, RB], F32)
            nc.vector.tensor_scalar(out=e1[:], in0=a_t[:], scalar1=-2.0,
                                    scalar2=1.0, op0=OP.mult, op1=OP.add)
            e2 = pb.tile([P, RB], F32)
            nc.vector.tensor_tensor(out=e2[:], in0=e1[:], in1=b_t[:],
                                    op=OP.subtract)
            jstar = pb.tile([P, RB], F32)
            nc.vector.tensor_tensor(out=jstar[:], in0=jcc[:], in1=e2[:],
                                    op=OP.add)

            # neighbors, pred-window start, flat element offsets
            jm = pb.tile([P, RB], F32)
            nc.vector.tensor_scalar(out=jm[:], in0=jstar[:], scalar1=-1.0,
                                    scalar2=0.0, op0=OP.add, op1=OP.max)
            jp = pb.tile([P, RB], F32)
            nc.vector.tensor_scalar(out=jp[:], in0=jstar[:], scalar1=1.0,
                                    scalar2=float(N - 1), op0=OP.add, op1=OP.min)
            s2 = pb.tile([P, RB], F32)
            nc.vector.tensor_scalar(out=s2[:], in0=jm[:], scalar1=float(N - 3),
                                    scalar2=None, op0=OP.min)
            s2i = pb.tile([P, RB], I32)
            nc.vector.tensor_copy(out=s2i[:], in_=s2[:])
            rowbase = pb.tile([P, RB], I32)  # (p*RB + q) * N
            nc.gpsimd.iota(rowbase[:], pattern=[[N, RB]], base=0,
                           channel_multiplier=RB * N)
            offs = pb.tile([P, RB], I32)
            nc.vector.tensor_tensor(out=offs[:], in0=rowbase[:], in1=s2i[:],
                                    op=OP.add)
            pw = pb.tile([P, RB * 3], F32)
            for q in range(RB):
                nc.gpsimd.indirect_dma_start(
                    out=pw[:, 3 * q:3 * q + 3], out_offset=None, in_=pred[:, :],
                    in_offset=bass.IndirectOffsetOnAxis(
                        ap=offs[:, q:q + 1], axis=1),
                )

            # in-window positions (0/1/2) of jstar, jm, jp
            p0 = pb.tile([P, RB], F32)
            nc.vector.tensor_tensor(out=p0[:], in0=jstar[:], in1=s2[:],
                                    op=OP.subtract)
            pmp = pb.tile([P, RB], F32)
            nc.vector.tensor_tensor(out=pmp[:], in0=jm[:], in1=s2[:],
                                    op=OP.subtract)
            ppp = pb.tile([P, RB], F32)
            nc.vector.tensor_tensor(out=ppp[:], in0=jp[:], in1=s2[:],
                                    op=OP.subtract)

            iota12 = pb.tile([P, RB * 3], F32)
            nc.gpsimd.iota(iota12[:], pattern=[[0, RB], [1, 3]], base=0,
                           channel_multiplier=0,
                           allow_small_or_imprecise_dtypes=True)

            # f(c): one-hot select of window position jstar
            m0 = pb.tile([P, RB * 3], F32)
            nc.vector.tensor_tensor(out=view3(m0), in0=view3(iota12),
                                    in1=p0[:].to_broadcast([P, RB, 3]),
                                    op=OP.is_equal)
            pr0 = pb.tile([P, RB * 3], F32)
            nc.vector.tensor_tensor(out=pr0[:], in0=m0[:], in1=pw[:],
                                    op=OP.mult)
            fpc = pb.tile([P, RB], F32)
            nc.vector.reduce_sum(out=fpc[:], in_=view3(pr0),
                                 axis=mybir.AxisListType.X)

            # f'(c): (pred[jp] - pred[jm]) / ((jp-jm)*dx) via +/- one-hot
            mp_ = pb.tile([P, RB * 3], F32)
            nc.vector.tensor_tensor(out=view3(mp_), in0=view3(iota12),
                                    in1=ppp[:].to_broadcast([P, RB, 3]),
                                    op=OP.is_equal)
            mm_ = pb.tile([P, RB * 3], F32)
            nc.vector.tensor_tensor(out=view3(mm_), in0=view3(iota12),
                                    in1=pmp[:].to_broadcast([P, RB, 3]),
                                    op=OP.is_equal)
            wd = pb.tile([P, RB * 3], F32)
            nc.vector.tensor_tensor(out=wd[:], in0=mp_[:], in1=mm_[:],
                                    op=OP.subtract)
            prd = pb.tile([P, RB * 3], F32)
            nc.vector.tensor_tensor(out=prd[:], in0=wd[:], in1=pw[:],
                                    op=OP.mult)
            df = pb.tile([P, RB], F32)
            nc.vector.reduce_sum(out=df[:], in_=view3(prd),
                                 axis=mybir.AxisListType.X)
            qd = pb.tile([P, RB], F32)
            nc.vector.tensor_tensor(out=qd[:], in0=jp[:], in1=jm[:],
                                    op=OP.subtract)
            den = pb.tile([P, RB], F32)
            nc.vector.tensor_scalar(out=den[:], in0=qd[:], scalar1=dx_t[:, :1],
                                    scalar2=None, op0=OP.mult)
            rden = pb.tile([P, RB], F32)
            nc.vector.reciprocal(out=rden[:], in_=den[:])
            fpp = pb.tile([P, RB], F32)
            nc.vector.tensor_tensor(out=fpp[:], in0=df[:], in1=rden[:],
                                    op=OP.mult)

            if debug:
                dbt = pb.tile([P, 44], F32)
                nc.vector.tensor_copy(out=dbt[:, 0:12], in_=xw[:])
                nc.vector.tensor_copy(out=dbt[:, 12:24], in_=pw[:])
                nc.vector.tensor_copy(out=dbt[:, 24:28], in_=jstar[:])
                nc.vector.tensor_copy(out=dbt[:, 28:32], in_=s2[:])
                nc.vector.tensor_copy(out=dbt[:, 32:36], in_=fpc[:])
                nc.vector.tensor_copy(out=dbt[:, 36:40], in_=fpp[:])
                offf = pb.tile([P, RB], F32)
                nc.vector.tensor_copy(out=offf[:], in_=offs[:])
                nc.vector.tensor_copy(out=dbt[:, 40:44], in_=offf[:])
                nc.sync.dma_start(dbg[:, :], dbt[:])

            # per-partition sums of (f(c)-1)^2 and f'(c)^2
            fpm1 = pb.tile([P, RB], F32)
            nc.vector.tensor_scalar(out=fpm1[:], in0=fpc[:], scalar1=-1.0,
                                    scalar2=None, op0=OP.add)
            sq2 = pb.tile([P, RB], F32)
            p2 = pb.tile([P, 1], F32)
            nc.scalar.activation(out=sq2[:], in_=fpm1[:],
                                 func=mybir.ActivationFunctionType.Square,
                                 accum_out=p2[:])
            sq3 = pb.tile([P, RB], F32)
            p3 = pb.tile([P, 1], F32)
            nc.scalar.activation(out=sq3[:], in_=fpp[:],
                                 func=mybir.ActivationFunctionType.Square,
                                 accum_out=p3[:])
            nc.sync.dma_start(partials[:, 1:2], p2[:])
            nc.sync.dma_start(partials[:, 2:3], p3[:])

            # ================= phase A: stream sum((p-t)^2) =============
            parts1 = pb.tile([P, NT], F32)
            k = 0
            for rb in range(BL // P):
                for ct in range(N // FT):
                    rs, cs = rb * P, ct * FT
                    pt = ppool.tile([P, FT], F32)
                    tt = tpool.tile([P, FT], F32)
                    nc.sync.dma_start(pt[:], pred[rs:rs + P, cs:cs + FT])
                    nc.sync.dma_start(tt[:], targ[rs:rs + P, cs:cs + FT])
                    dt = dpool.tile([P, FT], F32)
                    nc.vector.tensor_tensor(out=dt[:], in0=pt[:], in1=tt[:],
                                            op=OP.subtract)
                    # dt <- dt^2 in place; accum_out = row-sum
                    nc.scalar.activation(
                        out=dt[:], in_=dt[:],
                        func=mybir.ActivationFunctionType.Square,
                        accum_out=parts1[:, k:k + 1],
                    )
                    k += 1

            p1 = pb.tile([P, 1], F32)
            nc.vector.reduce_sum(out=p1[:], in_=parts1[:],
                                 axis=mybir.AxisListType.X)
            nc.sync.dma_start(partials[:, 0:1], p1[:])

    return nc


_NC_CACHE = None


def _get_nc():
    global _NC_CACHE
    if _NC_CACHE is None:
        nc = build_nc()
        # Bacc runs its compile pipeline (register alloc, sync-wait
        # splitting) in finalize; the PJRT exec path requires it.
        nc.finalize()
        _NC_CACHE = nc
    return _NC_CACHE


def make_in_maps(predicted_solution_batch, target_solution_batch,
                 c_input_batch, x_eval_points):
    pred = np.ascontiguousarray(predicted_solution_batch, dtype=np.float32)
    targ = np.ascontiguousarray(target_solution_batch, dtype=np.float32)
    c = np.ascontiguousarray(c_input_batch, dtype=np.float32)
    x = np.ascontiguousarray(x_eval_points, dtype=np.float32)
    dx = np.float32(x[1]) - np.float32(x[0])
    dxb = np.full((P, 1), dx, dtype=np.float32)
    xev = x.reshape(N, 1)
    in_maps = []
    for i in range(NCORES):
        sl = slice(i * BL, (i + 1) * BL)
        in_maps.append({
            "pred": pred[sl],
            "targ": targ[sl],
            "cvec": c[sl].reshape(P, RB),
            "xev": xev,
            "dxb": dxb,
        })
    return in_maps


def reduce_partials(results):
    s = np.zeros(3, dtype=np.float64)
    for r in results:
        s += r["partials"].astype(np.float64).sum(axis=0)
    loss = s[0] / (B * N) + s[1] / B + s[2] / B
    return np.float32(loss)


def kernel(predicted_solution_batch, target_solution_batch,
           c_input_batch, x_eval_points):
    nc = _get_nc()
    in_maps = make_in_maps(predicted_solution_batch, target_solution_batch,
                           c_input_batch, x_eval_points)
    res = run_bass_kernel_spmd(nc, in_maps, core_ids=list(range(NCORES)))
    return reduce_partials(res.results)
